# revision 1
# baseline (speedup 1.0000x reference)
"""Trainium2 Bass kernel for nn_EquivariantDeepSetsEncoder.

Strategy: data-parallel over batch (B=8) across 8 NeuronCores; one batch per
core. Per core the full 2048x2048 attention matrix E = exp(-pairwise_dist)
stays resident in SBUF (8 MB in bf16) and is reused by all three
message-passing layers, so HBM traffic is just the tiny inputs/outputs.

Key algebraic simplifications (exact, not approximations):
  * pairwise distances depend only on coordinate differences, so the centroid
    subtraction cancels inside the attention; E is built from raw x.
  * -dist(i,j) = x_i.x_j - |x_i|^2/2 - |x_j|^2/2 (then exp with scale=2),
    which a single augmented matmul computes.
  * softmax row-normalization (1/rowsum) cancels through LayerNorm's scale
    invariance because the pre-LN bias b_i is zero, so it is never computed.
  * LayerNorm mean comes for free as an extra (negated, pre-averaged) column
    of the layer weight matrix.

Precision: fp32 matmuls stream at 4 cycles/column on the PE, bf16 at 1. The
attention logits need ~fp32 accuracy, so they use a hi/lo split-precision
bf16 matmul (x = xh + xl, keeping the xh*xh + xh*xl + xl*xh terms, same for
the squared norms; K grows 5 -> 13 but moving cycles drop 4x). E, h and W
are plain bf16 (their quantization error averages out over 2048 points);
LayerNorm statistics and activations are computed in fp32.
"""

import math
import os

import numpy as np
import ml_dtypes

import concourse.bass as bass
import concourse.bacc as bacc
import concourse.mybir as mybir
import concourse.tile as tile
from concourse.bass_utils import run_bass_kernel_spmd
from concourse.vector_clock import ScopedClock

F32 = mybir.dt.float32
BF16 = mybir.dt.bfloat16
U32 = mybir.dt.uint32
AF = mybir.ActivationFunctionType
OP = mybir.AluOpType

B, N, D = 8, 2048, 3
P, R = 128, 16          # N = P * R; point (p, r) = original index 16*p + r
HID = (64, 128, 256)
LAT = 128
EPS = 1e-6
RSQRT_MAGIC = 0x5F3759DF


# ---------------------------------------------------------------------------
# Workaround for a walrus codegen limit in this toolchain: a NO_STRUCT
# instruction (Drain) can carry at most one sync-wait command. Tile's exit
# path attaches the full global-clock wait set to a single drain; split the
# waits across several drains instead. (Bacc's later legalization handles
# the rest of the instructions.)
def _split_drain_and_barrier(self, tick_clock, wait_clock):
    nc = self.nc
    drain_inst = nc.sync.drain()
    wait_clock.add_sem_waits(
        drain_inst.ins, ScopedClock({None: tick_clock.global_clock})
    )
    si = drain_inst.ins.sync_info
    waits = list(si.on_wait) if si is not None else []
    if len(waits) > 1:
        si.on_wait = [waits[0]]
        for w in waits[1:]:
            d2 = nc.sync.drain()
            if d2.ins.sync_info is not None:
                d2.ins.sync_info.on_wait = [w]
            else:
                d2.ins.sync_info = mybir.SyncInfo(on_wait=[w], on_update=[])
    nc.all_engine_barrier()
    assert self.sems is not None
    popped = nc._tile_sem_poison_stack.pop()
    assert popped is self._sem_poison
    nc.clear_and_free_semaphores(list(self.sems.allocated().values()))
    nc.all_engine_barrier()


def _apply_tile_patch():
    if os.environ.get("NO_DRAIN_PATCH", "0") == "1":
        return
    tile.TileContext._drain_and_barrier = _split_drain_and_barrier


# ---------------------------------------------------------------------------
def _emit_rsqrt(nc, out_ap, var_ap, w_t, t1_t, d_out):
    """out = sqrt(d_out / (var_ap + d_out*EPS)) == 1/sqrt(var + EPS), where
    var_ap holds sum-of-squares (d_out * var). Fast-inverse-sqrt seed plus
    three Newton iterations, fp32-accurate; DVE only (no activation table)."""
    nc.vector.tensor_single_scalar(out=w_t, in_=var_ap, scalar=d_out * EPS, op=OP.add)
    w_u = w_t.bitcast(U32)
    t1_u = t1_t.bitcast(U32)
    # seed bits = MAGIC - (w_bits >> 1). The DVE ALU is fp32 for +/-, so the
    # subtraction happens in float on the integer VALUES (result stays in
    # [5e8, 1.6e9], no wrap) and the uint32 output cast restores the bits;
    # the ~1e-5 relative bit noise is far below the seed's 3.4% error.
    nc.vector.tensor_scalar(
        out=t1_u, in0=w_u, scalar1=1, scalar2=None, op0=OP.logical_shift_right,
    )
    r_t = out_ap
    r_u = r_t.bitcast(U32)
    nc.vector.tensor_scalar(
        out=r_u, in0=t1_u, scalar1=-1.0, scalar2=float(RSQRT_MAGIC),
        op0=OP.mult, op1=OP.add,
    )
    sqd = math.sqrt(float(d_out))
    for it in range(2):
        nc.vector.tensor_tensor(out=t1_t, in0=r_t, in1=r_t, op=OP.mult)
        nc.vector.tensor_tensor(out=t1_t, in0=t1_t, in1=w_t, op=OP.mult)
        nc.vector.tensor_scalar(
            out=t1_t, in0=t1_t, scalar1=-0.5, scalar2=1.5, op0=OP.mult, op1=OP.add
        )
        if it < 1:
            nc.vector.tensor_tensor(out=r_t, in0=r_t, in1=t1_t, op=OP.mult)
        else:
            # fold the sqrt(d_out) factor into the final Newton multiply
            nc.vector.scalar_tensor_tensor(
                out=r_t, in0=r_t, scalar=sqd, in1=t1_t, op0=OP.mult, op1=OP.mult
            )


def _build(reps=1):
    nc = bacc.Bacc()
    xt = nc.dram_tensor("xt", [D, N], F32, kind="ExternalInput")
    xb = nc.dram_tensor("xb", [P, R * D], F32, kind="ExternalInput")
    mk = nc.dram_tensor("mk", [P, R], F32, kind="ExternalInput")
    w0a = nc.dram_tensor("w0a", [D, 2 * (HID[0] + 1)], BF16, kind="ExternalInput")
    w1a = nc.dram_tensor("w1a", [HID[0], 2 * (HID[1] + 1)], BF16, kind="ExternalInput")
    w2a = nc.dram_tensor("w2a", [HID[1], 2 * (HID[2] + 1)], BF16, kind="ExternalInput")
    wz2 = nc.dram_tensor("wz2", [P, 4 * LAT], BF16, kind="ExternalInput")
    bzt = nc.dram_tensor("bzt", [P, 1], F32, kind="ExternalInput")
    zout = nc.dram_tensor("z", [P, 1], F32, kind="ExternalOutput")
    cent_d = nc.dram_tensor("cent_scratch", [1, D], F32, kind="Internal")

    with tile.TileContext(nc) as tc:
        with tc.tile_pool(name="persist", bufs=1) as pp, \
             tc.tile_pool(name="scr", bufs=2) as scr:
            E_all = pp.tile([P, R * N], BF16, name="E_all")
            U13 = pp.tile([36, N], BF16, name="U13")
            V13 = pp.tile([36, N], BF16, name="V13")
            xtf = pp.tile([D, N], F32, name="xtf")
            xh3 = pp.tile([D, N], BF16, name="xh3")
            xl3 = pp.tile([D, N], BF16, name="xl3")
            xsq = pp.tile([D, N], F32, name="xsq")
            sneg = pp.tile([1, N], F32, name="sneg")
            shl = pp.tile([1, N], BF16, name="shl")
            sll = pp.tile([1, N], BF16, name="sll")
            onesb2 = pp.tile([2, N], BF16, name="onesb2")
            h0 = pp.tile([P, R * D], BF16, name="h0")
            h1 = pp.tile([P, R * HID[0]], BF16, name="h1")
            h2 = pp.tile([P, R * HID[1]], BF16, name="h2")
            h3 = pp.tile([P, R * HID[2]], BF16, name="h3")
            y_all = pp.tile([P, R * HID[2]], F32, name="y_all")
            EhT = pp.tile([P, N], BF16, name="EhT")
            xb_s = pp.tile([P, R * D], F32, name="xb_s")
            mk_s = pp.tile([P, R], F32, name="mk_s")
            msc = pp.tile([P, R], F32, name="msc")
            msc_b = pp.tile([P, R], BF16, name="msc_b")
            crow = pp.tile([1, R * D], F32, name="crow")
            w0_s = pp.tile([D, 2 * (HID[0] + 1)], BF16, name="w0_s")
            w1_s = pp.tile([HID[0], 2 * (HID[1] + 1)], BF16, name="w1_s")
            w2_s = pp.tile([HID[1], 2 * (HID[2] + 1)], BF16, name="w2_s")
            wz_s = pp.tile([P, 4 * LAT], BF16, name="wz_s")
            gfl_b = pp.tile([P, 2], BF16, name="gfl_b")
            bz_s = pp.tile([P, 1], F32, name="bz_s")
            ones31 = pp.tile([D, 1], F32, name="ones31")
            ones128 = pp.tile([P, 1], F32, name="ones128")
            ones1r = pp.tile([1, P], F32, name="ones1r")
            mkr = pp.tile([P, 1], F32, name="mkr")
            cnt_sb = pp.tile([1, 1], F32, name="cnt_sb")
            invc1 = pp.tile([1, 1], F32, name="invc1")
            invc_sb = pp.tile([P, 1], F32, name="invc_sb")
            cent_sb = pp.tile([D, 1], F32, name="cent_sb")
            varN = pp.tile([P, R], F32, name="varN")
            rstd = pp.tile([P, R], F32, name="rstd")
            rs_w = pp.tile([P, 8], F32, name="rs_w")
            rs_t1 = pp.tile([P, 8], F32, name="rs_t1")
            gf_b = pp.tile([P, 2], BF16, name="gf_b")
            z_sb = pp.tile([P, 1], F32, name="z_sb")

            for _rep in range(reps):
                # ------------- front: loads, hi/lo U/V build, centroid -------
                nc.sync.dma_start(out=xtf, in_=xt[:, :])
                nc.scalar.dma_start(out=xb_s, in_=xb[:, :])
                nc.scalar.dma_start(out=mk_s, in_=mk[:, :])
                nc.gpsimd.dma_start(out=w0_s, in_=w0a[:, :])
                nc.gpsimd.dma_start(out=w1_s, in_=w1a[:, :])
                nc.gpsimd.dma_start(out=w2_s, in_=w2a[:, :])
                nc.gpsimd.dma_start(out=wz_s, in_=wz2[:, :])
                nc.gpsimd.dma_start(out=bz_s, in_=bzt[:, :])
                nc.vector.memset(onesb2, 1.0)
                # rows 9..31 stay zero and contribute nothing to the K=36 matmul
                nc.vector.memset(U13, 0.0)
                nc.vector.memset(V13, 0.0)
                nc.gpsimd.memset(ones31, 1.0)
                nc.gpsimd.memset(ones128, 1.0)
                nc.gpsimd.memset(ones1r, 1.0)
                # dummy exp: pulls the ~2.7us exp table load into the
                # front's DMA window instead of stalling phase 1
                warm = pp.tile([1, 1], F32, name="warm")
                nc.vector.memset(warm, 0.0)
                nc.scalar.activation(out=warm, in_=warm, func=AF.Exp)
                # hi/lo split of the coordinates
                nc.vector.tensor_copy(out=xh3, in_=xtf)
                nc.vector.tensor_tensor(out=xl3, in0=xtf, in1=xh3, op=OP.subtract)
                nc.scalar.activation(out=xsq, in_=xtf, func=AF.Square)

                with tc.tile_pool(name="fpsum", bufs=1, space="PSUM") as fp:
                    sqp = fp.tile([1, N], F32, name="sqp")
                    for g in range(4):
                        nc.tensor.matmul(
                            sqp[:, 512 * g:512 * (g + 1)], lhsT=ones31,
                            rhs=xsq[:, 512 * g:512 * (g + 1)], start=True, stop=True,
                        )
                    nc.vector.tensor_scalar_mul(out=sneg, in0=sqp, scalar1=-0.5)
                    nc.vector.tensor_copy(out=shl, in_=sneg)
                    nc.vector.tensor_tensor(out=sll, in0=sneg, in1=shl, op=OP.subtract)
                    # engines only address partition starts {0,32,64,96}; DMA
                    # places single rows at arbitrary partitions.
                    # U rows: xh xh xl | -sqh/2 -sql/2 | 1 1
                    # V rows: xh xl xh |   1     1     | -sqh/2 -sql/2
                    nc.scalar.dma_start(out=U13[0:3, :], in_=xh3)
                    nc.scalar.dma_start(out=U13[3:6, :], in_=xh3)
                    nc.scalar.dma_start(out=U13[6:9, :], in_=xl3)
                    nc.scalar.dma_start(out=U13[32:33, :], in_=shl)
                    nc.scalar.dma_start(out=U13[33:34, :], in_=sll)
                    nc.scalar.dma_start(out=U13[34:36, :], in_=onesb2)
                    nc.sync.dma_start(out=V13[0:3, :], in_=xh3)
                    nc.sync.dma_start(out=V13[3:6, :], in_=xl3)
                    nc.sync.dma_start(out=V13[6:9, :], in_=xh3)
                    nc.sync.dma_start(out=V13[32:34, :], in_=onesb2)
                    nc.sync.dma_start(out=V13[34:35, :], in_=shl)
                    nc.sync.dma_start(out=V13[35:36, :], in_=sll)

                    # centroid = sum(x*m)/max(count,1); count = sum(m)
                    nc.vector.reduce_sum(out=mkr, in_=mk_s, axis=mybir.AxisListType.X)
                    cntp = fp.tile([1, 1], F32, name="cntp")
                    nc.tensor.matmul(cntp, lhsT=mkr, rhs=ones128, start=True, stop=True)
                    nc.vector.tensor_scalar_max(out=cnt_sb, in0=cntp, scalar1=1.0)
                    nc.vector.reciprocal(out=invc1, in_=cnt_sb)
                    invb = fp.tile([P, 1], F32, name="invb")
                    nc.tensor.matmul(invb, lhsT=ones1r, rhs=invc1, start=True, stop=True)
                    nc.vector.tensor_copy(out=invc_sb, in_=invb)
                    nc.vector.tensor_scalar_mul(out=msc, in0=mk_s, scalar1=invc_sb)
                    nc.vector.tensor_copy(out=msc_b, in_=msc)
                    centp = fp.tile([D, 1], F32, name="centp")
                    for r in range(R):
                        nc.tensor.matmul(
                            centp, lhsT=xb_s[:, D * r:D * (r + 1)], rhs=msc[:, r:r + 1],
                            start=(r == 0), stop=(r == R - 1),
                        )
                    nc.vector.tensor_copy(out=cent_sb, in_=centp)
                    nc.gpsimd.dma_start(out=cent_d[:, :], in_=cent_sb)
                    cent_ap = cent_d[:, :]
                    cbc = bass.AP(
                        tensor=cent_ap.tensor, offset=cent_ap.offset,
                        ap=[[0, 1], [0, R], [1, D]],
                    )
                    nc.gpsimd.dma_start(out=crow, in_=cbc)
                    c48p = fp.tile([P, R * D], F32, name="c48p")
                    nc.tensor.matmul(c48p, lhsT=ones1r, rhs=crow, start=True, stop=True)
                    nc.vector.tensor_tensor(out=h0, in0=xb_s, in1=c48p, op=OP.subtract)

                # ------------- phases 1+2 share the PSUM budget --------------
                with tc.tile_pool(name="spsum", bufs=2, space="PSUM") as sp, \
                     tc.tile_pool(name="pal0", bufs=1, space="PSUM") as pl0:
                    # phase 1: E = exp(-dist), 16 row-blocks of [128, 2048].
                    # Layer-0's (E @ h0) aggregation rides along: its four
                    # 512-col groups live at partition offsets 32g of ONE
                    # psum bank (d_in=3), so each E tile is consumed by PE
                    # right after its exp, hidden under the ACT-bound phase.
                    pa0 = pl0.tile([P, 512], F32, name="pa0")
                    for i in range(R):
                        for t in range(2):
                            ps = sp.tile([P, 1024], F32, name="ps", tag="ps")
                            for gg in range(2):
                                j0 = 1024 * t + 512 * gg
                                nc.tensor.matmul(
                                    ps[:, 512 * gg:512 * (gg + 1)],
                                    lhsT=U13[0:36, P * i:P * (i + 1)],
                                    rhs=V13[0:36, j0:j0 + 512],
                                    start=True, stop=True,
                                )
                            nc.scalar.activation(
                                out=E_all[:, N * i + 1024 * t: N * i + 1024 * (t + 1)],
                                in_=ps, func=AF.Exp, scale=2.0,
                            )
                        for g in range(4):
                            nc.tensor.matmul(
                                pa0[32 * g:32 * g + D, :],
                                lhsT=h0[:, D * i:D * (i + 1)],
                                rhs=E_all[:, N * i + 512 * g: N * i + 512 * (g + 1)],
                                start=(i == 0), stop=(i == R - 1),
                                tile_position=(0, 32 * g),
                            )
                    for g in range(4):
                        nc.vector.tensor_copy(
                            out=EhT[:D, 512 * g:512 * (g + 1)],
                            in_=pa0[32 * g:32 * g + D, :],
                        )

                with tc.tile_pool(name="apsum", bufs=3, space="PSUM") as apl, \
                     tc.tile_pool(name="bpsum", bufs=3, space="PSUM") as bpl:

                    # phase 2: three message-passing layers
                    layers = [
                        (h0, D, w0_s, HID[0], h1),
                        (h1, HID[0], w1_s, HID[1], h2),
                        (h2, HID[1], w2_s, HID[2], h3),
                    ]
                    for li, (hin, d_in, w_s, d_out, hout) in enumerate(layers):
                        # (E @ h)^T accumulated over the 16 point-chunks
                        # (layer 0's aggregation already ran under phase 1)
                        for g in range(4) if li > 0 else ():
                            pa = apl.tile([P, 512], F32, name="pa", tag="pa")
                            for r in range(R):
                                nc.tensor.matmul(
                                    pa[:d_in, :], lhsT=hin[:, d_in * r:d_in * (r + 1)],
                                    rhs=E_all[:, N * r + 512 * g: N * r + 512 * (g + 1)],
                                    start=(r == 0), stop=(r == R - 1),
                                )
                            nc.vector.tensor_copy(
                                out=EhT[:d_in, 512 * g:512 * (g + 1)], in_=pa[:d_in, :]
                            )
                        # @W_aug, center, variance, rsqrt, scale, swish
                        for half in range(2):
                            for c in range(8 * half, 8 * half + 8):
                                pb = bpl.tile([P, d_out + 1], F32, name="pb", tag="pb")
                                ehc = EhT[:d_in, P * c:P * (c + 1)]
                                nc.tensor.matmul(
                                    pb, lhsT=ehc, rhs=w_s[:, 0:d_out + 1],
                                    start=True, stop=False,
                                )
                                nc.tensor.matmul(
                                    pb, lhsT=ehc,
                                    rhs=w_s[:, d_out + 1:2 * (d_out + 1)],
                                    start=False, stop=True,
                                )
                                ysl = y_all[:, d_out * c:d_out * (c + 1)]
                                # y0 = u - mean(u)   (psum col d_out holds -mean)
                                nc.vector.tensor_scalar(
                                    out=ysl, in0=pb[:, :d_out],
                                    scalar1=pb[:, d_out:d_out + 1], scalar2=None,
                                    op0=OP.add,
                                )
                                sqo = scr.tile([P, d_out], F32, name="sqo", tag="sqo")
                                nc.scalar.activation(
                                    out=sqo, in_=ysl, func=AF.Square,
                                    accum_out=varN[:, c:c + 1],
                                )
                            h8 = slice(8 * half, 8 * half + 8)
                            _emit_rsqrt(
                                nc, rstd[:, h8], varN[:, h8], rs_w, rs_t1, d_out
                            )
                            for c in range(8 * half, 8 * half + 8):
                                ysl = y_all[:, d_out * c:d_out * (c + 1)]
                                nc.vector.tensor_scalar_mul(
                                    out=ysl, in0=ysl, scalar1=rstd[:, c:c + 1]
                                )
                            yhalf = y_all[:, d_out * 8 * half:d_out * 8 * (half + 1)]
                            hhalf = hout[:, d_out * 8 * half:d_out * 8 * (half + 1)]
                            nc.scalar.activation(out=hhalf, in_=yhalf, func=AF.Silu)

                # ------------- phase 3: masked mean pool + readout -----------
                with tc.tile_pool(name="tpsum", bufs=1, space="PSUM") as tp:
                    gf0 = tp.tile([P, 1], F32, name="gf0")
                    gf1 = tp.tile([P, 1], F32, name="gf1")
                    for t, gft in enumerate((gf0, gf1)):
                        for c in range(R):
                            o = HID[2] * c + P * t
                            nc.tensor.matmul(
                                gft, lhsT=h3[:, o:o + P], rhs=msc_b[:, c:c + 1],
                                start=(c == 0), stop=(c == R - 1),
                            )
                    nc.vector.tensor_copy(out=gf_b[:, 0:1], in_=gf0)
                    nc.vector.tensor_copy(out=gf_b[:, 1:2], in_=gf1)
                    nc.vector.tensor_tensor(out=gfl_b[:, 0:1], in0=gf0,
                                            in1=gf_b[:, 0:1], op=OP.subtract)
                    nc.vector.tensor_tensor(out=gfl_b[:, 1:2], in0=gf1,
                                            in1=gf_b[:, 1:2], op=OP.subtract)
                    zps = tp.tile([P, 1], F32, name="zps")
                    # wz_s columns: [wzh half0 | wzh half1 | wzl half0 | wzl half1]
                    # z ~= Wzh.gfh + Wzl.gfh + Wzh.gfl   (drop Wzl.gfl)
                    zmm = [(0, gf_b, 0), (1, gf_b, 1), (2, gf_b, 0), (3, gf_b, 1),
                           (0, gfl_b, 0), (1, gfl_b, 1)]
                    for k, (wcol, gsrc, gcol) in enumerate(zmm):
                        nc.tensor.matmul(
                            zps, lhsT=wz_s[:, LAT * wcol:LAT * (wcol + 1)],
                            rhs=gsrc[:, gcol:gcol + 1],
                            start=(k == 0), stop=(k == len(zmm) - 1),
                        )
                    nc.vector.scalar_tensor_tensor(
                        out=z_sb, in0=zps, scalar=1.0, in1=bz_s,
                        op0=OP.mult, op1=OP.add,
                    )
                    nc.sync.dma_start(out=zout[:, :], in_=z_sb)
    return nc


_NC_CACHE = None


def _get_nc():
    global _NC_CACHE
    if _NC_CACHE is None:
        _apply_tile_patch()
        nc = _build()
        nc.finalize()   # Bacc.compile(): wait legalization + register alloc
        _NC_CACHE = nc
    return _NC_CACHE


def _host_prep(inputs):
    x = np.asarray(inputs["x"], np.float32)
    mask = np.asarray(inputs["mask"], np.float32)
    W = [np.asarray(inputs[f"W{i}"], np.float32) for i in range(3)]
    Wz = np.asarray(inputs["Wz"], np.float32)
    bz = np.asarray(inputs["bz"], np.float32)

    def hilo(a):
        hi = a.astype(ml_dtypes.bfloat16)
        lo = (a - hi.astype(np.float32)).astype(ml_dtypes.bfloat16)
        return hi, lo

    waug = []
    for i in range(3):
        a = np.concatenate([W[i], -W[i].mean(axis=1, keepdims=True)], axis=1)
        hi, lo = hilo(a)
        waug.append(np.ascontiguousarray(np.concatenate([hi, lo], axis=1)))
    wzflat = np.concatenate([Wz[:P, :], Wz[P:, :]], axis=1)
    wzh, wzl = hilo(wzflat)
    wz2 = np.ascontiguousarray(np.concatenate([wzh, wzl], axis=1))
    bzr = np.ascontiguousarray(bz.reshape(P, 1))

    in_maps = []
    for bi in range(B):
        in_maps.append({
            "xt": np.ascontiguousarray(
                x[bi].reshape(P, R, D).transpose(2, 1, 0).reshape(D, N)
            ),
            "xb": np.ascontiguousarray(x[bi].reshape(P, R * D)),
            "mk": np.ascontiguousarray(mask[bi].reshape(P, R)),
            "w0a": waug[0], "w1a": waug[1], "w2a": waug[2],
            "wz2": wz2, "bzt": bzr,
        })
    return in_maps


def kernel(**inputs):
    for i in range(3):
        if (np.any(np.asarray(inputs[f"b{i}"])) or
                np.any(np.asarray(inputs[f"be{i}"])) or
                np.any(np.asarray(inputs[f"g{i}"]) != 1.0)):
            raise NotImplementedError(
                "kernel specialized for zero LN/layer biases and unit gains"
            )
    in_maps = _host_prep(inputs)
    nc = _get_nc()
    res = run_bass_kernel_spmd(nc, in_maps, core_ids=list(range(B)))
    return np.stack([res.results[i]["z"][:, 0] for i in range(B)]).astype(np.float32)



# revision 19
# speedup vs baseline: 1.6998x; 1.6998x over previous
"""Trainium2 Bass kernel for nn_EquivariantDeepSetsEncoder.

Strategy: data-parallel over batch (B=8) across 8 NeuronCores; one batch per
core. Per core the full 2048x2048 attention matrix E = exp(-pairwise_dist)
stays resident in SBUF (4 MB in fp8e4) and is reused by all three
message-passing layers, so HBM traffic is just the tiny inputs/outputs.

Key algebraic simplifications (exact, not approximations):
  * pairwise distances depend only on coordinate differences, so the centroid
    subtraction cancels inside the attention; E is built from raw x.
  * -dist(i,j) = x_i.x_j - |x_i|^2/2 - |x_j|^2/2 (then exp with scale=2),
    which a single augmented matmul computes.
  * softmax row-normalization (1/rowsum) cancels through LayerNorm's scale
    invariance because the pre-LN bias b_i is zero, so it is never computed.
  * LayerNorm mean comes for free as an extra (negated, pre-averaged) column
    of the layer weight matrix.

Precision/engine layout:
  * E and the per-layer h activations are fp8e4 so the three aggregation
    passes run as DoubleRow matmuls (half-rate cycles per output row, 256
    contraction per instruction).  Logits are built from a hi/lo bf16 split
    (fp32-accurate); E's fp8 quantization noise averages out over 2048
    points and the masked mean pool.
  * LayerNorm statistics: centered y0 is evicted PSUM->SBUF (DVE or ACT,
    per-chunk choice), sum-of-squares comes from a fused DVE
    scalar_tensor_tensor with accumulator, rstd via Newton rsqrt on DVE.
  * A tunable subset of E row-chunks computes exp on DVE with a one-op
    Schraudolph bit-trick (bf16 output), offloading the ACT engine.
"""

import math
import os

import numpy as np
import ml_dtypes

import concourse.bass as bass
import concourse.bacc as bacc
import concourse.mybir as mybir
import concourse.tile as tile
from concourse.bass_utils import run_bass_kernel_spmd
from concourse.vector_clock import ScopedClock

F32 = mybir.dt.float32
BF16 = mybir.dt.bfloat16
F8 = mybir.dt.float8e4
F32R = mybir.dt.float32r
U32 = mybir.dt.uint32
U16 = mybir.dt.uint16
AF = mybir.ActivationFunctionType
OP = mybir.AluOpType
DR = mybir.MatmulPerfMode.DoubleRow

B, N, D = 8, 2048, 3
P, R = 128, 16          # N = P * R; point (p, r) = original index 16*p + r
HID = (64, 128, 256)
LAT = 128
EPS = 1e-6
RSQRT_MAGIC = 0x5F3759DF

# Row-chunk pairs whose exp runs on DVE (Schraudolph, bf16 E) instead of
# ACT (fp8 E).  Pairs are (2i, 2i+1); value = set of pair indices 0..7.
DVE_EXP_PAIRS = frozenset(int(x) for x in os.environ.get(
    "DVE_EXP_PAIRS", "").split(",") if x != "")
# Schraudolph constants for bf16 output: bits16 = L*A16 + B16, truncated.
# L = psum logit (=-dist/2), exp(2L) ~ 2^(2L*log2e).
_A16 = 2.0 * 1.4426950408889634 * 128.0
_B16 = 127.0 * 128.0 + 0.5 - 0.0579 * 128.0  # -C tuned for min rel err

# (layer*16+chunk) ids whose y0 eviction / y-scale run on ACT instead of
# DVE (Identity with bias/scale APs), to balance engine load in phase 2
_dflt_y0 = ",".join(str(li * 16 + c) for li in (1, 2) for c in range(3, 16, 4))
ACT_Y0_CHUNKS = frozenset(int(x) for x in os.environ.get(
    "ACT_Y0_CHUNKS", _dflt_y0).split(",") if x != "")
_dflt_y = ",".join(str(li * 16 + c) for li in (1, 2) for c in range(0, 16, 2))
ACT_Y_CHUNKS = frozenset(int(x) for x in os.environ.get(
    "ACT_Y_CHUNKS", _dflt_y).split(",") if x != "")


# ---------------------------------------------------------------------------
# Workaround for a walrus codegen limit in this toolchain: a NO_STRUCT
# instruction (Drain) can carry at most one sync-wait command. Tile's exit
# path attaches the full global-clock wait set to a single drain; split the
# waits across several drains instead.
def _split_drain_and_barrier(self, tick_clock, wait_clock):
    nc = self.nc
    drain_inst = nc.sync.drain()
    wait_clock.add_sem_waits(
        drain_inst.ins, ScopedClock({None: tick_clock.global_clock})
    )
    si = drain_inst.ins.sync_info
    waits = list(si.on_wait) if si is not None else []
    if len(waits) > 1:
        si.on_wait = [waits[0]]
        for w in waits[1:]:
            d2 = nc.sync.drain()
            if d2.ins.sync_info is not None:
                d2.ins.sync_info.on_wait = [w]
            else:
                d2.ins.sync_info = mybir.SyncInfo(on_wait=[w], on_update=[])
    nc.all_engine_barrier()
    assert self.sems is not None
    popped = nc._tile_sem_poison_stack.pop()
    assert popped is self._sem_poison
    nc.clear_and_free_semaphores(list(self.sems.allocated().values()))
    nc.all_engine_barrier()


def _apply_tile_patch():
    if os.environ.get("NO_DRAIN_PATCH", "0") == "1":
        return
    tile.TileContext._drain_and_barrier = _split_drain_and_barrier


# ---------------------------------------------------------------------------
def _emit_rsqrt(nc, out_ap, var_ap, w_t, t1_t, d_out):
    """out = sqrt(d_out / (var_ap + d_out*EPS)) == 1/sqrt(var + EPS), where
    var_ap holds sum-of-squares (d_out * var). Fast-inverse-sqrt seed plus
    two Newton iterations, DVE only (no activation table)."""
    nc.vector.tensor_single_scalar(out=w_t, in_=var_ap, scalar=d_out * EPS, op=OP.add)
    w_u = w_t.bitcast(U32)
    t1_u = t1_t.bitcast(U32)
    nc.vector.tensor_scalar(
        out=t1_u, in0=w_u, scalar1=1, scalar2=None, op0=OP.logical_shift_right,
    )
    r_t = out_ap
    r_u = r_t.bitcast(U32)
    nc.vector.tensor_scalar(
        out=r_u, in0=t1_u, scalar1=-1.0, scalar2=float(RSQRT_MAGIC),
        op0=OP.mult, op1=OP.add,
    )
    sqd = math.sqrt(float(d_out))
    for it in range(2):
        nc.vector.tensor_tensor(out=t1_t, in0=r_t, in1=r_t, op=OP.mult)
        nc.vector.tensor_tensor(out=t1_t, in0=t1_t, in1=w_t, op=OP.mult)
        nc.vector.tensor_scalar(
            out=t1_t, in0=t1_t, scalar1=-0.5, scalar2=1.5, op0=OP.mult, op1=OP.add
        )
        if it < 1:
            nc.vector.tensor_tensor(out=r_t, in0=r_t, in1=t1_t, op=OP.mult)
        else:
            nc.vector.scalar_tensor_tensor(
                out=r_t, in0=r_t, scalar=sqd, in1=t1_t, op0=OP.mult, op1=OP.mult
            )


def _build(reps=1):
    nc = bacc.Bacc()
    xt = nc.dram_tensor("xt", [D, N], F32, kind="ExternalInput")
    xb = nc.dram_tensor("xb", [P, R * D], F32, kind="ExternalInput")
    mk = nc.dram_tensor("mk", [P, R], F32, kind="ExternalInput")
    w0a = nc.dram_tensor("w0a", [D, 2 * (HID[0] + 1)], F8, kind="ExternalInput")
    w1a = nc.dram_tensor("w1a", [HID[0], 2 * (HID[1] + 1)], F8, kind="ExternalInput")
    w2a = nc.dram_tensor("w2a", [HID[1], 2 * (HID[2] + 1)], F8, kind="ExternalInput")
    wz2 = nc.dram_tensor("wz2", [P, 4 * LAT], BF16, kind="ExternalInput")
    bzt = nc.dram_tensor("bzt", [P, 1], F32, kind="ExternalInput")
    zout = nc.dram_tensor("z", [P, 1], F32, kind="ExternalOutput")
    cent_d = nc.dram_tensor("cent_scratch", [1, D], F32, kind="Internal")

    n_dve = len(DVE_EXP_PAIRS)
    dve_pairs = sorted(DVE_EXP_PAIRS)
    f8_pairs = [i for i in range(8) if i not in DVE_EXP_PAIRS]

    with tile.TileContext(nc) as tc:
        with tc.tile_pool(name="persist", bufs=1) as pp, \
             tc.tile_pool(name="scr", bufs=2) as scr:
            # E storage: fp8 chunks (ACT exp) and bf16 chunks (DVE exp).
            # Chunk-pair members sit in adjacent N-col blocks so DoubleRow
            # k-tile APs ([P][2][cols], stride N) slice straight out.
            E_f8 = pp.tile([P, max(16 - 2 * n_dve, 1) * N], F8, name="E_f8")
            E_bf = (pp.tile([P, 2 * n_dve * N], BF16, name="E_bf")
                    if n_dve else None)
            # U/V rows live at quadrant partition bases so engines write
            # them directly (no SBUF->SBUF placement DMAs):
            #   U: [0:3]=xh [32:35]=xh [64:67]=xl [96]=-|x|^2hi/2 [97:99]=1
            #   V: [0:3]=xh [32:35]=xl [64:67]=xh [96]=1 [97]=-hi/2 [98]=-lo/2
            # The row-side norm uses only the bf16-hi part: the resulting
            # per-row scale exp(-|x_i|^2_hi) cancels exactly in LayerNorm.
            U13 = pp.tile([99, N], BF16, name="U13")
            V13 = pp.tile([99, N], BF16, name="V13")
            xtf = pp.tile([D, N], F32, name="xtf")
            xsq = pp.tile([D, N], F32, name="xsq")
            sll = pp.tile([1, N], BF16, name="sll")
            onesb = pp.tile([1, N], BF16, name="onesb")
            # h0 padded to 64 dims/chunk: DoubleRow stationary tiles need
            # M >= 64 to pass the ISA check; columns 3..63 stay zero
            h0 = pp.tile([P, R * 64], F8, name="h0")
            h1 = pp.tile([P, R * HID[0]], F8, name="h1")
            h2 = pp.tile([P, R * HID[1]], F8, name="h2")
            h3 = pp.tile([P, R * HID[2]], BF16, name="h3")
            y0_all = pp.tile([P, R * HID[2]], BF16, name="y0_all")
            y_sil = pp.tile([P, R * HID[2]], BF16, name="y_sil")
            sqd_scr = pp.tile([P, HID[2]], BF16, name="sqd_scr")
            EhT = pp.tile([P, N], F8, name="EhT")
            xb_s = pp.tile([P, R * D], F32, name="xb_s")
            mk_s = pp.tile([P, R], F32, name="mk_s")
            msc = pp.tile([P, R], F32, name="msc")
            msc_b = pp.tile([P, R], BF16, name="msc_b")
            crow = pp.tile([1, R * D], F32, name="crow")
            w0_s = pp.tile([D, 2 * (HID[0] + 1)], F8, name="w0_s")
            w1_s = pp.tile([HID[0], 2 * (HID[1] + 1)], F8, name="w1_s")
            w2_s = pp.tile([HID[1], 2 * (HID[2] + 1)], F8, name="w2_s")
            wz_s = pp.tile([P, 4 * LAT], BF16, name="wz_s")
            gfl_b = pp.tile([P, 2], BF16, name="gfl_b")
            bz_s = pp.tile([P, 1], F32, name="bz_s")
            ones31 = pp.tile([D, 1], F32, name="ones31")
            ones128 = pp.tile([P, 1], F32, name="ones128")
            ones1r = pp.tile([1, P], F32, name="ones1r")
            mkr = pp.tile([P, 1], F32, name="mkr")
            cnt_sb = pp.tile([1, 1], F32, name="cnt_sb")
            invc1 = pp.tile([1, 1], F32, name="invc1")
            invc_sb = pp.tile([P, 1], F32, name="invc_sb")
            cent_sb = pp.tile([D, 1], F32, name="cent_sb")
            varN = pp.tile([P, R], F32, name="varN")
            mneg = pp.tile([P, R], F32, name="mneg")
            rstd = pp.tile([P, R], F32, name="rstd")
            rs_w = pp.tile([P, 8], F32, name="rs_w")
            rs_t1 = pp.tile([P, 8], F32, name="rs_t1")
            gf_b = pp.tile([P, 2], BF16, name="gf_b")
            z_sb = pp.tile([P, 1], F32, name="z_sb")

            # one-time init of persistent zero/one regions (unwritten U/V
            # rows must stay zero; the ones rows never change)
            nc.vector.memset(U13, 0.0)
            nc.vector.memset(V13, 0.0)
            nc.vector.memset(onesb, 1.0)
            nc.vector.memset(V13[96:97, :], 1.0)
            nc.sync.dma_start(out=U13[97:98, :], in_=onesb)
            nc.sync.dma_start(out=U13[98:99, :], in_=onesb)
            nc.gpsimd.memset(ones31, 1.0)
            nc.gpsimd.memset(h0, 0.0)
            nc.gpsimd.memset(ones128, 1.0)
            nc.gpsimd.memset(ones1r, 1.0)

            def pair3(ap2, width):
                """[P, 2*width] AP -> [P][2][width] k-tile AP (stride width)."""
                return bass.AP(
                    tensor=ap2.tensor, offset=ap2.offset,
                    ap=[ap2.ap[0], [width, 2], [1, width]],
                )

            def E_ap(i, j0, j1):
                """E chunk i, logical col slice [j0:j1) -> storage AP."""
                if i // 2 in DVE_EXP_PAIRS:
                    ii = 2 * dve_pairs.index(i // 2) + (i % 2)
                    return E_bf[:, ii * N + j0:ii * N + j1]
                ii = 2 * f8_pairs.index(i // 2) + (i % 2)
                return E_f8[:, ii * N + j0:ii * N + j1]

            def E_pair(pi, j0, j1):
                """DoubleRow rhs AP for chunk pair pi: [P][2][j1-j0]."""
                ii = 2 * f8_pairs.index(pi)
                base = E_f8[:, ii * N + j0:ii * N + j1]
                return bass.AP(
                    tensor=base.tensor, offset=base.offset,
                    ap=[base.ap[0], [N, 2], [1, j1 - j0]],
                )

            def dup2(ap2):
                """[K, M] AP -> [K][2][M] with stride-0 k-tile duplication."""
                return bass.AP(
                    tensor=ap2.tensor, offset=ap2.offset,
                    ap=[ap2.ap[0], [0, 2], ap2.ap[1]],
                )

            def h_pair(h, pi, d):
                base = h[:, 2 * pi * d:(2 * pi + 1) * d]
                return bass.AP(
                    tensor=base.tensor, offset=base.offset,
                    ap=[base.ap[0], [d, 2], [1, d]],
                )

            for _rep in range(reps):
                # ------------- front: loads, hi/lo U/V build, centroid -------
                nc.sync.dma_start(out=xtf, in_=xt[:, :])
                nc.scalar.dma_start(out=xb_s, in_=xb[:, :])
                nc.scalar.dma_start(out=mk_s, in_=mk[:, :])
                nc.gpsimd.dma_start(out=w0_s, in_=w0a[:, :])
                nc.gpsimd.dma_start(out=w1_s, in_=w1a[:, :])
                nc.gpsimd.dma_start(out=bz_s, in_=bzt[:, :])
                # dummy exp: pulls the exp table load into the front's DMA
                # window instead of stalling phase 1
                warm = pp.tile([1, 1], F32, name="warm")
                nc.vector.memset(warm, 0.0)
                nc.scalar.activation(out=warm, in_=warm, func=AF.Exp)

                with tc.tile_pool(name="fpsum", bufs=1, space="PSUM") as fp:
                    # centroid chain first: its DVE/PE ops must not queue
                    # behind the U/V build work (engine queues are in-order)
                    nc.vector.reduce_sum(out=mkr, in_=mk_s, axis=mybir.AxisListType.X)
                    cntp = fp.tile([1, 1], F32, name="cntp")
                    nc.tensor.matmul(cntp, lhsT=mkr, rhs=ones128, start=True, stop=True)
                    nc.vector.tensor_scalar_max(out=cnt_sb, in0=cntp, scalar1=1.0)
                    nc.vector.reciprocal(out=invc1, in_=cnt_sb)
                    invb = fp.tile([P, 1], F32, name="invb")
                    nc.tensor.matmul(invb, lhsT=ones1r, rhs=invc1, start=True, stop=True)
                    nc.vector.tensor_copy(out=invc_sb, in_=invb)
                    nc.vector.tensor_scalar_mul(out=msc, in0=mk_s, scalar1=invc_sb)
                    nc.vector.tensor_copy(out=msc_b, in_=msc)
                    centp = fp.tile([D, 1], F32, name="centp")
                    for r in range(R):
                        nc.tensor.matmul(
                            centp, lhsT=xb_s[:, D * r:D * (r + 1)], rhs=msc[:, r:r + 1],
                            start=(r == 0), stop=(r == R - 1),
                        )
                    nc.vector.tensor_copy(out=cent_sb, in_=centp)
                    nc.gpsimd.dma_start(out=cent_d[:, :], in_=cent_sb)
                    cent_ap = cent_d[:, :]
                    cbc = bass.AP(
                        tensor=cent_ap.tensor, offset=cent_ap.offset,
                        ap=[[0, 1], [0, R], [1, D]],
                    )
                    nc.gpsimd.dma_start(out=crow, in_=cbc)
                    c48p = fp.tile([P, R * D], F32, name="c48p")
                    nc.tensor.matmul(c48p, lhsT=ones1r, rhs=crow, start=True, stop=True)
                    # h0 = xc in fp8 (strided 3-of-64 write; rest stays 0)
                    h0w = bass.AP(
                        tensor=h0[:, 0:1].tensor, offset=h0[:, 0:1].offset,
                        ap=[h0[:, 0:1].ap[0], [64, R], [1, D]],
                    )
                    nc.vector.tensor_tensor(
                        out=h0w, in0=xb_s, in1=c48p, op=OP.subtract)

                    # U/V build: coordinate hi part straight into U, square
                    # for the norms, lo parts / replications on DVE
                    nc.vector.tensor_tensor(out=xsq, in0=xtf, in1=xtf, op=OP.mult)
                    nc.scalar.activation(out=U13[0:3, :], in_=xtf, func=AF.Copy)
                    nc.vector.tensor_tensor(
                        out=V13[32:35, :], in0=xtf, in1=U13[0:3, :], op=OP.subtract)
                    nc.vector.tensor_copy(out=V13[0:3, :], in_=U13[0:3, :])
                    nc.vector.tensor_copy(out=U13[32:35, :], in_=U13[0:3, :])
                    nc.vector.tensor_copy(out=V13[64:67, :], in_=U13[0:3, :])
                    nc.vector.tensor_copy(out=U13[64:67, :], in_=V13[32:35, :])

                    sqp = fp.tile([1, N], F32, name="sqp")
                    for g in range(4):
                        nc.tensor.matmul(
                            sqp[:, 512 * g:512 * (g + 1)], lhsT=ones31,
                            rhs=xsq[:, 512 * g:512 * (g + 1)],
                            start=True, stop=True,
                        )
                    # hi part of -|x|^2/2 directly into U row 96; lo residual
                    # via one fused DVE op; the two V-side rows (non-quadrant
                    # partitions 97/98) go through tiny single-row DMAs.
                    nc.scalar.activation(
                        out=U13[96:97, :], in_=sqp, func=AF.Copy, scale=-0.5)
                    nc.vector.scalar_tensor_tensor(
                        out=sll, in0=sqp, scalar=-0.5, in1=U13[96:97, :],
                        op0=OP.mult, op1=OP.subtract,
                    )
                    nc.sync.dma_start(out=V13[97:98, :], in_=U13[96:97, :])
                    nc.sync.dma_start(out=V13[98:99, :], in_=sll)
                    # big weight loads issued only now so the V-row DMAs
                    # (which gate phase 1) don't queue behind them
                    nc.sync.dma_start(out=w2_s, in_=w2a[:, :])
                    nc.sync.dma_start(out=wz_s, in_=wz2[:, :])

                # ------------- phases 1+2 share the PSUM budget --------------
                with tc.tile_pool(name="spsum", bufs=2, space="PSUM") as sp, \
                     tc.tile_pool(name="pal0", bufs=1, space="PSUM") as pl0:
                    # phase 1: E = exp(-dist), 16 row-blocks of [128, 2048].
                    # Layer-0's (E @ h0) aggregation rides along in one psum
                    # bank (d_in=3, four 512-col groups at partition offsets
                    # 32g), consumed right after each pair's exp.
                    pa0s = [pl0.tile([P, 512], F32, name=f"pa0_{g}")
                            for g in range(4)]
                    pair_order = dve_pairs + f8_pairs
                    for po, pi in enumerate(pair_order):
                        for half in range(4):
                            i = 2 * pi + half // 2
                            t = half % 2
                            ps = sp.tile([P, 1024], F32, name="ps", tag="ps")
                            for gg in range(2):
                                j0 = 1024 * t + 512 * gg
                                nc.tensor.matmul(
                                    ps[:, 512 * gg:512 * (gg + 1)],
                                    lhsT=U13[0:99, P * i:P * (i + 1)],
                                    rhs=V13[0:99, j0:j0 + 512],
                                    start=True, stop=True,
                                )
                            if pi in DVE_EXP_PAIRS:
                                # Schraudolph exp2 on DVE: bf16 bits as int
                                eap = E_ap(i, 1024 * t, 1024 * (t + 1))
                                nc.vector.tensor_scalar(
                                    out=eap.bitcast(U16), in0=ps,
                                    scalar1=_A16, scalar2=_B16,
                                    op0=OP.mult, op1=OP.add,
                                )
                            else:
                                nc.scalar.activation(
                                    out=E_ap(i, 1024 * t, 1024 * (t + 1)),
                                    in_=ps, func=AF.Exp, scale=2.0,
                                )
                        # aggregate pair pi into pa0 (DoubleRow for fp8)
                        for g in range(4):
                            if pi in DVE_EXP_PAIRS:
                                for k in range(2):
                                    i = 2 * pi + k
                                    nc.tensor.matmul(
                                        pa0s[g][0:D, :],
                                        lhsT=h0[:, i * 64:i * 64 + D],
                                        rhs=E_ap(i, 512 * g, 512 * (g + 1)),
                                        start=(po == 0 and k == 0),
                                        stop=(po == 7 and k == 1),
                                    )
                            else:
                                nc.tensor.matmul(
                                    pa0s[g][0:64, :],
                                    lhsT=h_pair(h0, pi, 64),
                                    rhs=E_pair(pi, 512 * g, 512 * (g + 1)),
                                    start=(po == 0), stop=(po == 7),
                                    perf_mode=DR,
                                )
                    # 1/16 keeps unnormalized E@h inside fp8e4 range; the
                    # scale cancels in LayerNorm so nothing compensates it
                    for g in range(4):
                        nc.scalar.activation(
                            out=EhT[:D, 512 * g:512 * (g + 1)],
                            in_=pa0s[g][0:D, :], func=AF.Copy, scale=1.0 / 16,
                        )

                with tc.tile_pool(name="apsum", bufs=3, space="PSUM") as apl, \
                     tc.tile_pool(name="bpsum", bufs=3, space="PSUM") as bpl:

                    # phase 2: three message-passing layers
                    layers = [
                        (h0, D, 64, w0_s, HID[0], h1),
                        (h1, HID[0], 64, w1_s, HID[1], h2),
                        (h2, HID[1], 128, w2_s, HID[2], h3),
                    ]
                    for li, (hin, d_in, hst, w_s, d_out, hout) in enumerate(layers):
                        # (E @ h)^T accumulated over the 8 chunk-pairs
                        # (layer 0's aggregation already ran under phase 1)
                        for g in range(4) if li > 0 else ():
                            pa = apl.tile([P, 512], F32, name="pa", tag="pa")
                            for po, pi in enumerate(range(8)):
                                if pi in DVE_EXP_PAIRS:
                                    for k in range(2):
                                        i = 2 * pi + k
                                        nc.tensor.matmul(
                                            pa[:d_in, :],
                                            lhsT=hin[:, i * hst:i * hst + d_in],
                                            rhs=E_ap(i, 512 * g, 512 * (g + 1)),
                                            start=(po == 0 and k == 0),
                                            stop=(po == 7 and k == 1),
                                        )
                                else:
                                    nc.tensor.matmul(
                                        pa[:hst, :],
                                        lhsT=h_pair(hin, pi, hst),
                                        rhs=E_pair(pi, 512 * g, 512 * (g + 1)),
                                        start=(po == 0), stop=(po == 7),
                                        perf_mode=DR,
                                    )
                            nc.scalar.activation(
                                out=EhT[:d_in, 512 * g:512 * (g + 1)],
                                in_=pa[:d_in, :], func=AF.Copy, scale=1.0 / 16,
                            )
                        # @W_aug, center, variance, rsqrt, scale, swish
                        # processed in quarters (4 chunks) so rsqrt barriers
                        # stay small and h completes incrementally
                        for q in range(4):
                            for c in range(4 * q, 4 * q + 4):
                                pb = bpl.tile([P, d_out + 1], F32, name="pb", tag="pb")
                                ehc = EhT[:d_in, P * c:P * (c + 1)]
                                nc.tensor.matmul(
                                    pb, lhsT=ehc, rhs=w_s[:, 0:d_out + 1],
                                    start=True, stop=False,
                                )
                                nc.tensor.matmul(
                                    pb, lhsT=ehc,
                                    rhs=w_s[:, d_out + 1:2 * (d_out + 1)],
                                    start=False, stop=True,
                                )
                                y0c = y0_all[:, d_out * c:d_out * (c + 1)]
                                # y0 = u - mean(u)   (psum col d_out = -mean)
                                if li * 16 + c in ACT_Y0_CHUNKS:
                                    nc.vector.tensor_copy(
                                        out=mneg[:, c:c + 1],
                                        in_=pb[:, d_out:d_out + 1],
                                    )
                                    nc.scalar.activation(
                                        out=y0c, in_=pb[:, :d_out],
                                        func=AF.Identity,
                                        bias=mneg[:, c:c + 1],
                                    )
                                else:
                                    nc.vector.tensor_scalar(
                                        out=y0c, in0=pb[:, :d_out],
                                        scalar1=pb[:, d_out:d_out + 1],
                                        scalar2=None, op0=OP.add,
                                    )
                                # sum of squares (bf16 4x mode)
                                nc.vector.scalar_tensor_tensor(
                                    out=sqd_scr[:, :d_out], in0=y0c, scalar=1.0,
                                    in1=y0c, op0=OP.mult, op1=OP.mult,
                                    accum_out=varN[:, c:c + 1],
                                )
                            q4 = slice(4 * q, 4 * q + 4)
                            _emit_rsqrt(
                                nc, rstd[:, q4], varN[:, q4],
                                rs_w[:, 0:4], rs_t1[:, 0:4], d_out,
                            )
                            for c in range(4 * q, 4 * q + 4):
                                if li * 16 + c in ACT_Y_CHUNKS:
                                    nc.scalar.activation(
                                        out=y_sil[:, d_out * c:d_out * (c + 1)],
                                        in_=y0_all[:, d_out * c:d_out * (c + 1)],
                                        func=AF.Identity,
                                        scale=rstd[:, c:c + 1],
                                    )
                                else:
                                    nc.vector.tensor_scalar_mul(
                                        out=y_sil[:, d_out * c:d_out * (c + 1)],
                                        in0=y0_all[:, d_out * c:d_out * (c + 1)],
                                        scalar1=rstd[:, c:c + 1],
                                    )
                            yq = y_sil[:, d_out * 4 * q:d_out * 4 * (q + 1)]
                            hh = hout[:, d_out * 4 * q:d_out * 4 * (q + 1)]
                            nc.scalar.activation(out=hh, in_=yq, func=AF.Silu)

                # ------------- phase 3: masked mean pool + readout -----------
                with tc.tile_pool(name="tpsum", bufs=1, space="PSUM") as tp:
                    gf0 = tp.tile([P, 1], F32, name="gf0")
                    gf1 = tp.tile([P, 1], F32, name="gf1")
                    for t, gft in enumerate((gf0, gf1)):
                        for c in range(R):
                            o = HID[2] * c + P * t
                            nc.tensor.matmul(
                                gft, lhsT=h3[:, o:o + P], rhs=msc_b[:, c:c + 1],
                                start=(c == 0), stop=(c == R - 1),
                            )
                    nc.vector.tensor_copy(out=gf_b[:, 0:1], in_=gf0)
                    nc.vector.tensor_copy(out=gf_b[:, 1:2], in_=gf1)
                    nc.vector.tensor_tensor(out=gfl_b[:, 0:1], in0=gf0,
                                            in1=gf_b[:, 0:1], op=OP.subtract)
                    nc.vector.tensor_tensor(out=gfl_b[:, 1:2], in0=gf1,
                                            in1=gf_b[:, 1:2], op=OP.subtract)
                    zps = tp.tile([P, 1], F32, name="zps")
                    # wz_s columns: [wzh half0 | wzh half1 | wzl half0 | wzl half1]
                    # z ~= Wzh.gfh + Wzl.gfh + Wzh.gfl   (drop Wzl.gfl)
                    zmm = [(0, gf_b, 0), (1, gf_b, 1), (2, gf_b, 0), (3, gf_b, 1),
                           (0, gfl_b, 0), (1, gfl_b, 1)]
                    for k, (wcol, gsrc, gcol) in enumerate(zmm):
                        nc.tensor.matmul(
                            zps, lhsT=wz_s[:, LAT * wcol:LAT * (wcol + 1)],
                            rhs=gsrc[:, gcol:gcol + 1],
                            start=(k == 0), stop=(k == len(zmm) - 1),
                        )
                    nc.vector.scalar_tensor_tensor(
                        out=z_sb, in0=zps, scalar=1.0, in1=bz_s,
                        op0=OP.mult, op1=OP.add,
                    )
                    nc.sync.dma_start(out=zout[:, :], in_=z_sb)
    return nc


_NC_CACHE = None


def _get_nc():
    global _NC_CACHE
    if _NC_CACHE is None:
        _apply_tile_patch()
        nc = _build()
        nc.finalize()
        _NC_CACHE = nc
    return _NC_CACHE


def _host_prep(inputs):
    x = np.asarray(inputs["x"], np.float32)
    mask = np.asarray(inputs["mask"], np.float32)
    W = [np.asarray(inputs[f"W{i}"], np.float32) for i in range(3)]
    Wz = np.asarray(inputs["Wz"], np.float32)
    bz = np.asarray(inputs["bz"], np.float32)

    def hilo(a, dt=ml_dtypes.bfloat16):
        hi = a.astype(dt)
        lo = (a - hi.astype(np.float32)).astype(dt)
        return hi, lo

    waug = []
    for i in range(3):
        a = np.concatenate([W[i], -W[i].mean(axis=1, keepdims=True)], axis=1)
        hi, lo = hilo(a, ml_dtypes.float8_e4m3fn)
        waug.append(np.ascontiguousarray(np.concatenate([hi, lo], axis=1)))
    wzflat = np.concatenate([Wz[:P, :], Wz[P:, :]], axis=1)
    wzh, wzl = hilo(wzflat)
    wz2 = np.ascontiguousarray(np.concatenate([wzh, wzl], axis=1))
    bzr = np.ascontiguousarray(bz.reshape(P, 1))

    in_maps = []
    for bi in range(B):
        in_maps.append({
            "xt": np.ascontiguousarray(
                x[bi].reshape(P, R, D).transpose(2, 1, 0).reshape(D, N)
            ),
            "xb": np.ascontiguousarray(x[bi].reshape(P, R * D)),
            "mk": np.ascontiguousarray(mask[bi].reshape(P, R)),
            "w0a": waug[0], "w1a": waug[1], "w2a": waug[2],
            "wz2": wz2, "bzt": bzr,
        })
    return in_maps


def kernel(**inputs):
    for i in range(3):
        if (np.any(np.asarray(inputs[f"b{i}"])) or
                np.any(np.asarray(inputs[f"be{i}"])) or
                np.any(np.asarray(inputs[f"g{i}"]) != 1.0)):
            raise NotImplementedError(
                "kernel specialized for zero LN/layer biases and unit gains"
            )
    in_maps = _host_prep(inputs)
    nc = _get_nc()
    res = run_bass_kernel_spmd(nc, in_maps, core_ids=list(range(B)))
    return np.stack([res.results[i]["z"][:, 0] for i in range(B)]).astype(np.float32)
